# revision 4
# baseline (speedup 1.0000x reference)
"""LSTM decoder kernel for Trainium2, 8 NeuronCores.

Data-parallel over batch (32 rows/core, no collectives).  Per-core
matmuls are batch-major with 4x PE column tiling: each 32-wide col-tile
j computes a different 256-col window of the gate dim, so the four
concurrent streams reach full-array throughput despite M=32.

Per-step schedule (the recurrence h_t -> gates_{t+1} is the critical
path; everything else hides under the matmul streams):
- psum gate layout is split by hidden half: bank0 = i|f|g|o for columns
  0:128 of each col-tile, bank1 = the same for 128:256.  Each bank's
  32-MM accumulation group gets its own stop, so half-0's activation
  chain + transpose + t1-copy runs while bank1 is still streaming, and
  the next step's k-even matmuls (which need only t1) start right after.
- sigmoid via tanh: the host pre-scales the i,f,o columns of W_hh and
  x_proj by 0.5, so ONE tanh over each 512-wide bank produces
  ti'|tf'|g|to', and the 0.5*t+0.5 affine folds into fused DVE
  affine_mul_reduce ops:
    tmp = (ti*.5+.5)*g ; c = (tf*.5+.5)*c + tmp ; h = (to*.5+.5)*tanh(c)
- x_proj (constant across steps) is injected in bf16 via identity-
  stationary matmuls; y matmuls for step t-1 and the xp injection for
  step t+1 fill the PE window while the half-1 chain completes.
- cell state c stays fp32 in SBUF; h is re-transposed each step with
  two PE transposes (separate PSUM banks) into the next stationaries.
"""
import numpy as np
import ml_dtypes

import concourse.bass as bass
import concourse.mybir as mybir
import concourse.tile as tile
from concourse import bacc
from concourse import bass_utils

B, H, O, T, NCORES = 256, 1024, 512, 128, 8
BL = B // NCORES
BF16 = mybir.dt.bfloat16
F32 = mybir.dt.float32

_CACHE = {}

K_ORDER = [0, 2, 4, 6, 1, 3, 5, 7]  # k-even (t1) first


def _emit_bank_mms(nc, gates_ps, w_sb, t1, t2, bank):
    for k in K_ORDER:
        tt = t1 if k % 2 == 0 else t2
        stat = tt[:, 32 * (k // 2):32 * (k // 2) + 32]
        for j in range(4):
            nc.tensor.matmul(
                gates_ps[32 * j:32 * (j + 1), 512 * bank:512 * (bank + 1)],
                stat,
                w_sb[k][:, 2048 * bank + 512 * j:2048 * bank + 512 * (j + 1)],
                start=False,
                stop=(k == 7 and j == 3),
                tile_position=(0, 32 * j),
                skip_group_check=True,
            )


def _emit_xp_mms(nc, gates_ps, eyeb, xp_sb, final):
    for bank in range(2):
        for j in range(4):
            nc.tensor.matmul(
                gates_ps[32 * j:32 * (j + 1), 512 * bank:512 * (bank + 1)],
                eyeb[:, 32 * bank:32 * bank + 32],
                xp_sb[:, (bank * 4 + j) * 512:(bank * 4 + j + 1) * 512],
                start=True,
                stop=(final and j == 3),
                tile_position=(0, 32 * j),
                skip_group_check=True,
            )


def _emit_y_mms(nc, y_ps, wl_sb, t1, t2):
    for k in K_ORDER:
        tt = t1 if k % 2 == 0 else t2
        stat = tt[:, 32 * (k // 2):32 * (k // 2) + 32]
        for j in range(4):
            nc.tensor.matmul(
                y_ps[32 * j:32 * (j + 1), 0:128],
                stat,
                wl_sb[:, 512 * k + 128 * j:512 * k + 128 * j + 128],
                start=(k == 0),
                stop=(k == 7 and j == 3),
                tile_position=(0, 32 * j),
                skip_group_check=True,
            )


def _build(steps=T, late_tr=False):
    nc = bacc.Bacc("TRN2", target_bir_lowering=False, debug=False,
                   num_devices=NCORES)
    w_d = nc.dram_tensor("W", [128, 8 * 4096], BF16, kind="ExternalInput").ap()
    wl_d = nc.dram_tensor("Wl", [128, 4096], BF16, kind="ExternalInput").ap()
    xp_d = nc.dram_tensor("xp", [128, 4096], BF16, kind="ExternalInput").ap()
    eyeb_d = nc.dram_tensor("eyeb", [128, 128], BF16, kind="ExternalInput").ap()
    y_d = nc.dram_tensor("y", [T, 128, 128], F32, kind="ExternalOutput").ap()

    ACT = mybir.ActivationFunctionType
    mult = mybir.AluOpType.mult
    addop = mybir.AluOpType.add

    with tile.TileContext(nc) as tc:
        with tc.tile_pool(name="stat", bufs=1) as statp, \
             tc.tile_pool(name="sb", bufs=2) as sb, \
             tc.tile_pool(name="ps", bufs=2, space="PSUM") as ps, \
             tc.tile_pool(name="tp", bufs=1, space="PSUM") as tpp:
            w_sb = []
            for k in range(8):
                wk = statp.tile([128, 4096], BF16, tag=f"W{k}")
                nc.sync.dma_start(wk[:], w_d[:, 4096 * k:4096 * (k + 1)])
                w_sb.append(wk)
            wl_sb = statp.tile([128, 4096], BF16, tag="Wl")
            nc.sync.dma_start(wl_sb[:], wl_d)
            xp_sb = statp.tile([128, 4096], BF16, tag="xp")
            nc.sync.dma_start(xp_sb[:], xp_d)
            eyeb = statp.tile([128, 128], BF16, tag="eyeb")
            nc.sync.dma_start(eyeb[:], eyeb_d)
            c_sb = statp.tile([128, 256], F32, tag="c")
            nc.gpsimd.memset(c_sb[:], 0.0)

            t1_prev = t2_prev = None
            gates_cur = ps.tile([128, 1024], F32, tag="gates")
            _emit_xp_mms(nc, gates_cur, eyeb, xp_sb, final=True)

            for t in range(steps):
                if t > 0:
                    _emit_bank_mms(nc, gates_cur, w_sb, t1_prev, t2_prev, 0)
                    _emit_bank_mms(nc, gates_cur, w_sb, t1_prev, t2_prev, 1)

                tg = sb.tile([128, 1024], F32, tag="tg")
                th = sb.tile([128, 256], F32, tag="th")
                tmp = sb.tile([128, 256], F32, tag="tmp")
                acc = sb.tile([128, 8], F32, tag="acc")
                h_sb = sb.tile([128, 256], BF16, tag="h")
                tts = []
                if late_tr and t > 0:
                    y_ps = ps.tile([128, 512], F32, tag="y")
                    _emit_y_mms(nc, y_ps, wl_sb, t1_prev, t2_prev)
                    if t < steps - 1:
                        gates_next = ps.tile([128, 1024], F32, tag="gates")
                        _emit_xp_mms(nc, gates_next, eyeb, xp_sb, final=False)
                for x in (0, 1):
                    base = 512 * x
                    lo, hi = 128 * x, 128 * x + 128
                    # one tanh over the whole bank: ti'|tf'|g|to'
                    nc.scalar.activation(tg[:, base:base + 512],
                                         gates_cur[:, base:base + 512], ACT.Tanh)
                    ti = tg[:, base:base + 128]
                    tf = tg[:, base + 128:base + 256]
                    g = tg[:, base + 256:base + 384]
                    to = tg[:, base + 384:base + 512]
                    nc.vector.affine_mul_reduce(
                        tmp[:, lo:hi], acc[:, 4 * x:4 * x + 1], ti, g, 0.5, 0.5)
                    nc.vector.affine_mul_reduce(
                        c_sb[:, lo:hi], acc[:, 4 * x + 1:4 * x + 2], tf,
                        c_sb[:, lo:hi], 0.5, 0.5)
                    nc.vector.tensor_tensor(c_sb[:, lo:hi], c_sb[:, lo:hi],
                                            tmp[:, lo:hi], addop)
                    nc.scalar.activation(th[:, lo:hi], c_sb[:, lo:hi], ACT.Tanh)
                    nc.vector.affine_mul_reduce(
                        h_sb[:, lo:hi], acc[:, 4 * x + 2:4 * x + 3], to,
                        th[:, lo:hi], 0.5, 0.5)
                    tpx = tpp.tile([128, 1024], BF16, tag=f"tp{x}")
                    nc.tensor.transpose(tpx[:, 0:128], h_sb[:, lo:hi], eyeb[:])
                    tx = sb.tile([128, 128], BF16, tag=f"t{x + 1}")
                    nc.vector.tensor_copy(tx[:], tpx[:, 0:128])
                    tts.append(tx)

                    if (not late_tr) and x == 0 and t > 0:
                        y_ps = ps.tile([128, 512], F32, tag="y")
                        _emit_y_mms(nc, y_ps, wl_sb, t1_prev, t2_prev)
                        if t < steps - 1:
                            gates_next = ps.tile([128, 1024], F32, tag="gates")
                            _emit_xp_mms(nc, gates_next, eyeb, xp_sb,
                                         final=False)

                if t == 0 and steps > 1:
                    gates_next = ps.tile([128, 1024], F32, tag="gates")
                    _emit_xp_mms(nc, gates_next, eyeb, xp_sb, final=False)

                if t > 0:
                    y_sb = sb.tile([128, 128], F32, tag="ysb")
                    nc.scalar.activation(y_sb[:], y_ps[:, 0:128], ACT.Copy)
                    nc.sync.dma_start(y_d[(t - 1) % T], y_sb[:])

                t1_prev, t2_prev = tts
                if t < steps - 1:
                    gates_cur = gates_next

            y_ps = ps.tile([128, 512], F32, tag="y")
            _emit_y_mms(nc, y_ps, wl_sb, t1_prev, t2_prev)
            y_sb = sb.tile([128, 128], F32, tag="ysb")
            nc.scalar.activation(y_sb[:], y_ps[:, 0:128], ACT.Copy)
            nc.sync.dma_start(y_d[(steps - 1) % T], y_sb[:])

    nc.compile()
    return nc


def _colmap():
    """Device gate-col -> original col (psum: per bank, i|f|g|o 128-blocks)."""
    m = np.empty(4096, np.int64)
    ar = np.arange(128)
    for j in range(4):
        for bank in range(2):
            for q in range(4):
                d0 = 2048 * bank + 512 * j + 128 * q
                m[d0:d0 + 128] = q * 1024 + 256 * j + 128 * bank + ar
    return m


def _sig_scale():
    """0.5 on i,f,o device columns (sigmoid-via-tanh), 1.0 on g."""
    s = np.full(4096, 0.5, np.float32)
    d = np.arange(4096)
    s[(d % 512) // 128 == 2] = 1.0
    return s


def _prep_inputs(C, W_ih, W_hh, b_ih, b_hh, W_lin):
    xp = np.asarray(C, np.float32) @ np.asarray(W_ih, np.float32).T
    xp = xp + np.asarray(b_ih, np.float32) + np.asarray(b_hh, np.float32)
    cm = _colmap()
    ss = _sig_scale()
    w_perm = np.asarray(W_hh, np.float32).T[:, cm] * ss
    w_dev = np.ascontiguousarray(
        w_perm.reshape(8, 128, 4096)
        .transpose(1, 0, 2).reshape(128, 8 * 4096)).astype(ml_dtypes.bfloat16)
    wl_dev = np.ascontiguousarray(
        np.asarray(W_lin, np.float32).T.reshape(8, 128, 512)
        .transpose(1, 0, 2).reshape(128, 4096)).astype(ml_dtypes.bfloat16)
    eyeb = np.eye(128, dtype=ml_dtypes.bfloat16)
    in_maps = []
    for c in range(NCORES):
        xpb = xp[BL * c:BL * (c + 1)][:, cm] * ss
        xp_c = np.zeros((128, 4096), np.float32)
        for bank in range(2):
            xp_c[32 * bank:32 * (bank + 1), 2048 * bank:2048 * (bank + 1)] = \
                xpb[:, 2048 * bank:2048 * (bank + 1)]
        in_maps.append({"W": w_dev, "Wl": wl_dev,
                        "xp": xp_c.astype(ml_dtypes.bfloat16),
                        "eyeb": eyeb})
    return in_maps


def kernel(C, W_ih, W_hh, b_ih, b_hh, W_lin, b_lin, max_seq_len):
    assert int(max_seq_len) == T and C.shape == (B, H)
    if "nc" not in _CACHE:
        _CACHE["nc"] = _build()
    nc = _CACHE["nc"]
    in_maps = _prep_inputs(C, W_ih, W_hh, b_ih, b_hh, W_lin)
    try:
        res = bass_utils.run_bass_kernel_spmd(
            nc, in_maps, core_ids=list(range(NCORES)))
    except Exception:
        res = bass_utils.run_bass_kernel_spmd(
            nc, in_maps, core_ids=list(range(NCORES)))
    out = np.empty((T, B, O), np.float32)
    blin = np.asarray(b_lin, np.float32)
    for c in range(NCORES):
        yc = res.results[c]["y"]
        out[:, BL * c:BL * (c + 1), :] = (
            yc.reshape(T, 4, BL, 128).transpose(0, 2, 1, 3).reshape(T, BL, O)
            + blin)
    return out
